# revision 5
# baseline (speedup 1.0000x reference)
"""3-layer GCN (gcn_norm + 3x gather/scatter conv) on 8 TRN2 NeuronCores.

Strategy
--------
dst-shard nodes across the 8 cores (after a degree-sorted round-robin
relabeling that balances per-window edge counts across cores, keeping the
SPMD program shape identical on every core).

Per layer:
  * each core transforms its own node slice (X @ W, pre-scaled by
    dinv = deg^-1/2) and AllGathers the full [N, 64] feature matrix into
    every core's DRAM,
  * each core dma_gathers the source rows of its incident edges (sorted by
    destination window; 2 lists split by source half so gather indices fit
    in int16), one 256B row per edge slot,
  * segment-sums edge messages on the TensorEngine: per 128-row window pair,
    psum = I^T @ h'_own (self loops) + sum_chunks S_chunk^T @ M_chunk where
    S is a one-hot [128 edges x 64 rows] matrix generated on the VectorEngine
    by comparing dst-offsets against an iota tile,
  * eviction fuses the dinv post-scale and the activation.

Self-contained: only numpy + the concourse (Bass/Tile) stack available on
the machine. Does not read any sibling files.
"""
import math
import os
import sys
import types

import numpy as np

for _p in ("/opt/trn_rl_repo",):
    if _p not in sys.path and os.path.isdir(_p):
        sys.path.insert(0, _p)

import concourse.bacc as bacc
import concourse.bass as bass
import concourse.mybir as mybir
from concourse import tile
from concourse.bass_utils import run_bass_kernel_spmd

F32 = mybir.dt.float32

NCORES = 8
W = 128         # window rows (one S matrix column block)
NB = 32         # chunks per dma_gather batch
CH = 64         # hidden channels

# Perf results of the last kernel() call (populated when BASS_TRACE=1).
LAST_PERF = None


def _install_ntff_hook():
    """antenv.axon_hooks is absent on this image; recreate it so that
    run_bass_kernel_spmd(trace=True) can capture NTFF profiles."""
    if "antenv.axon_hooks" in sys.modules:
        return
    try:
        from trn_agent_boot.trn_boot import _ntff_profile_via_ctypes

        mod = types.ModuleType("antenv.axon_hooks")
        box = [None]
        mod.set_axon_ntff_profile_hook = lambda h: box.__setitem__(0, h)
        mod.get_axon_ntff_profile_hook = lambda: box[0]
        mod.set_axon_ntff_profile_hook(
            _ntff_profile_via_ctypes("/opt/axon/libaxon_pjrt.so")
        )
        sys.modules["antenv.axon_hooks"] = mod
    except Exception:
        pass


def _prep(x, edge_index):
    """Host-side graph preprocessing. Returns (meta, per_core_inputs, newid)."""
    N = x.shape[0]
    E = edge_index.shape[1]
    NPC = N // NCORES
    NW = -(-NPC // W)            # windows per core (= 128-row tiles)
    NPCp = NW * W                # padded nodes per core
    Np = NPCp * NCORES           # padded total nodes
    NT = NW                      # 128-row tiles per core

    src = np.ascontiguousarray(edge_index[0]).astype(np.int64)
    dst = np.ascontiguousarray(edge_index[1]).astype(np.int64)

    deg = 1.0 + np.bincount(dst, minlength=N).astype(np.float64)
    dinv = (1.0 / np.sqrt(deg)).astype(np.float32)

    # degree-sorted round-robin deal: rank r -> core (r//W)%8, window r//(W*8)
    order = np.argsort(-deg, kind="stable")          # rank -> old id
    r = np.arange(N)
    new_of_rank = ((r // W) % NCORES) * NPCp + (r // (W * NCORES)) * W + (r % W)
    newid = np.empty(N, np.int64)
    newid[order] = new_of_rank                       # old -> new

    PPC = ((NW + 1) // 2) * W                        # pair rows per core (3200)

    s_new = newid[src]
    d_new = newid[dst]
    core = d_new // NPCp
    dloc = d_new % NPCp
    win = dloc // W
    rel = (dloc % W).astype(np.int16)
    off = s_new % NPCp
    scre = s_new // NPCp
    wloc = off // W
    prt = off % W
    # pair row = same partition in adjacent windows (2k, 2k+1); the lone last
    # window self-pairs (its odd half is a duplicate). parity = window parity.
    pid = (scre * PPC + (wloc >> 1) * W + prt).astype(np.int16)
    lst = (wloc & 1).astype(np.int64)

    # per (core, win, list) counts -> shared chunk schedule (max over cores)
    cnt = np.zeros((NCORES, NW, 2), np.int64)
    np.add.at(cnt, (core, win, lst), 1)
    chmax = -(-cnt.max(axis=0) // 128)               # [NW, 2] chunks
    base = np.zeros((NW, 2), np.int64)               # chunk base within list
    Cl = [0, 0]
    for l in (0, 1):
        base[:, l] = np.cumsum(chmax[:, l]) - chmax[:, l]
        Cl[l] = int(chmax[:, l].sum())

    # slot assignment: edge -> (core, list, slot) with slot inside the
    # window's chunk range; vectorized cumcount within (core, list, win)
    key = (core * 2 + lst) * NW + win
    o = np.argsort(key, kind="stable")
    ks = key[o]
    grp_start = np.zeros(E, np.int64)
    new_grp = np.empty(E, np.bool_)
    new_grp[0] = True
    new_grp[1:] = ks[1:] != ks[:-1]
    starts = np.nonzero(new_grp)[0]
    grp_of = np.cumsum(new_grp) - 1
    cumcount = np.arange(E) - starts[grp_of]
    slot_sorted = base[win[o], lst[o]] * 128 + cumcount

    gidx = [np.zeros((NCORES, Cl[l] * 128), np.int16) for l in (0, 1)]
    drel = [np.full((NCORES, Cl[l] * 128), W, np.int16) for l in (0, 1)]
    for l in (0, 1):
        m = lst[o] == l
        gidx[l][core[o][m], slot_sorted[m]] = pid[o][m]
        drel[l][core[o][m], slot_sorted[m]] = rel[o][m]

    # sort slots within each 128-chunk by gather index: better HBM row
    # locality for the random 256B reads (any within-chunk order is valid,
    # the S one-hot handles the slot->dst mapping; pads keep drel=W)
    for l in (0, 1):
        g3 = gidx[l].reshape(NCORES, Cl[l], 128)
        d3 = drel[l].reshape(NCORES, Cl[l], 128)
        srt = np.argsort(g3, axis=2, kind="stable")
        gidx[l] = np.take_along_axis(g3, srt, axis=2).reshape(NCORES, -1)
        drel[l] = np.take_along_axis(d3, srt, axis=2).reshape(NCORES, -1)

    # packed layouts
    def pack_idx(a):  # [C*128] -> [128, C*8], idx i at [i%16, i//16], repl x8
        half = a.reshape(-1, 16).T                   # [16, C*8]
        return np.tile(half, (8, 1)).astype(np.int16)

    def pack_drel(a, C):  # [C*128] -> [128, C]
        return np.ascontiguousarray(a.reshape(C, 128).T)

    dinv_new = np.zeros(Np, np.float32)
    dinv_new[newid] = dinv
    x_new = np.zeros((Np, x.shape[1]), np.float32)
    x_new[newid] = x

    per_core = []
    for c in range(NCORES):
        d = {}
        d["idx0"] = pack_idx(gidx[0][c])
        d["idx1"] = pack_idx(gidx[1][c])
        d["drel0"] = pack_drel(drel[0][c], Cl[0])
        d["drel1"] = pack_drel(drel[1][c], Cl[1])
        d["dinv"] = np.ascontiguousarray(
            dinv_new[c * NPCp : (c + 1) * NPCp].reshape(NT, 128).T
        )
        d["xT"] = np.ascontiguousarray(x_new[c * NPCp : (c + 1) * NPCp].T)
        per_core.append(d)

    meta = dict(
        N=N, Np=Np, NPC=NPC, NPCp=NPCp, NW=NW, NT=NT, PPC=PPC,
        chmax=chmax, base=base, Cl=Cl, INC=x.shape[1],
    )
    return meta, per_core, newid


def _build(nc, meta, has_b1, has_b2, has_b3):
    """Trace the SPMD tile program for one core."""
    STAGE = int(os.environ.get("K_STAGE", "99"))
    NT, NW, Np, NPCp = meta["NT"], meta["NW"], meta["Np"], meta["NPCp"]
    Cl, chmax, base = meta["Cl"], meta["chmax"], meta["base"]
    PPC, INC = meta["PPC"], meta["INC"]
    BF = mybir.dt.bfloat16
    OUTC = 16

    # ---- I/O -----------------------------------------------------------
    xT_d = nc.dram_tensor("xT", [INC, NPCp], F32, kind="ExternalInput")
    idx_d = [
        nc.dram_tensor(f"idx{l}", [128, Cl[l] * 8], mybir.dt.int16,
                       kind="ExternalInput")
        for l in (0, 1)
    ]
    drel_d = [
        nc.dram_tensor(f"drel{l}", [128, Cl[l]], mybir.dt.int16,
                       kind="ExternalInput")
        for l in (0, 1)
    ]
    dinv_d = nc.dram_tensor("dinv", [128, NT], F32, kind="ExternalInput")
    w1_d = nc.dram_tensor("w1", [INC, CH], F32, kind="ExternalInput")
    w2_d = nc.dram_tensor("w2", [CH, CH], F32, kind="ExternalInput")
    w3_d = nc.dram_tensor("w3", [CH, OUTC], F32, kind="ExternalInput")
    ident_d = nc.dram_tensor("ident", [128, 128], F32, kind="ExternalInput")
    iota_d = nc.dram_tensor("iota", [128, W], mybir.dt.int16,
                            kind="ExternalInput")
    b_d = {}
    if has_b1:
        b_d[1] = nc.dram_tensor("b1b", [128, CH], F32, kind="ExternalInput")
    if has_b2:
        b_d[2] = nc.dram_tensor("b2b", [128, CH], F32, kind="ExternalInput")
    if has_b3:
        b_d[3] = nc.dram_tensor("b3b", [128, OUTC], F32, kind="ExternalInput")
    out_d = nc.dram_tensor("out", [128, NT * OUTC], F32, kind="ExternalOutput")

    with tile.TileContext(nc) as tc:
        with (
            tc.tile_pool(name="const", bufs=1) as cpool,
            tc.tile_pool(name="hp", bufs=2) as hp_pool,
            tc.tile_pool(name="act", bufs=2) as act_pool,
            tc.tile_pool(name="xt", bufs=2) as xt_pool,
            tc.tile_pool(name="stage", bufs=4) as stg_pool,
            tc.tile_pool(name="hpb", bufs=2) as hpb_pool,
            tc.tile_pool(name="msg0", bufs=2) as msg_pool0,
            tc.tile_pool(name="msg1", bufs=2) as msg_pool1,
            tc.tile_pool(name="sgen0", bufs=2) as s_pool0,
            tc.tile_pool(name="sgen1", bufs=2) as s_pool1,
            tc.tile_pool(name="aggps", bufs=4, space="PSUM") as agg_psum,
            tc.tile_pool(name="trps", bufs=2, space="PSUM") as tr_psum,
            tc.tile_pool(name="trxt", bufs=2, space="PSUM") as xt_psum,
            tc.tile_pool(name="dram", bufs=1, space="DRAM") as dram,
        ):
            # ---- residents ------------------------------------------------
            def load(shape, dtype, src):
                t = cpool.tile(shape, dtype, tag=f"c_{src.name}")
                nc.sync.dma_start(t[:], src[:])
                return t

            t_xT = load([INC, NPCp], F32, xT_d)
            t_idx = [load([128, Cl[l] * 8], mybir.dt.int16, idx_d[l]) for l in (0, 1)]
            t_drel = [load([128, Cl[l]], mybir.dt.int16, drel_d[l])
                      for l in (0, 1)]
            t_dinv = load([128, NT], F32, dinv_d)
            t_w1 = load([INC, CH], F32, w1_d)
            t_w2 = load([CH, CH], F32, w2_d)
            t_w3 = load([CH, OUTC], F32, w3_d)
            t_id = load([128, 128], F32, ident_d)
            t_iota = load([128, W], mybir.dt.int16, iota_d)
            t_b = {k: load(v.shape, F32, v) for k, v in b_d.items()}


            def dinv_ap(t):
                return t_dinv[:][:, t : t + 1]

            ag_bufs = []
            for i in range(3):
                agi = dram.tile([PPC, 128], BF, tag=f"ag_in{i}",
                                name=f"ag_in{i}")
                agf = dram.tile([PPC * NCORES, 128], BF, addr_space="Shared",
                                tag=f"ag_full{i}", name=f"ag_full{i}")
                ag_bufs.append((agi, agf))

            # ---- helpers --------------------------------------------------
            def allgather(i):
                ag_in, ag_full = ag_bufs[i]
                nc.gpsimd.collective_compute(
                    "AllGather",
                    mybir.AluOpType.bypass,
                    replica_groups=[list(range(NCORES))],
                    ins=[ag_in[:].opt()],
                    outs=[ag_full[:].opt()],
                )

            def store_hp(hpb, i):
                # hpb bf16 [128, NT*CH] -> pair-major DRAM [PPC, 128]:
                # row ((w>>1)*128+p) cols [s*64, ...), s = w&1.
                ag = ag_bufs[i][0]
                npair = (NT - 1) // 2
                dst = ag[:][0 : npair * W, :].rearrange(
                    "(t p) (s c) -> p t s c", p=W, s=2
                )
                srcv = hpb[:][:, 0 : 2 * npair * CH].rearrange(
                    "p (t s c) -> p t s c", s=2, c=CH
                )
                with nc.allow_non_contiguous_dma("pair store"):
                    nc.sync.dma_start(dst, srcv)
                for s in (0, 1):
                    nc.sync.dma_start(
                        ag[:][npair * W : (npair + 1) * W,
                              s * CH : (s + 1) * CH],
                        hpb[:][:, (NT - 1) * CH : NT * CH],
                    )

            def agg_layer(hp_tile, evict, i):
                """Gather + segment-sum for one layer.

                hp_tile: resident [128, NT*CH] self-term rows (pre-scaled).
                evict(t, psum_ap): emit eviction for window t.
                """
                ag_full = ag_bufs[i][1]
                src_ap = [ag_full[:], ag_full[:]]
                loaded = [0, 0]
                stiles = [[], []]
                htiles = [[], []]

                def ensure(l, j):
                    while j >= loaded[l] * NB:
                        b = loaded[l]
                        nb = min(NB, Cl[l] - b * NB)
                        q = (i * 97 + b * 2 + l) % 4
                        mpool = msg_pool0 if l == 0 else msg_pool1
                        spool = s_pool0 if l == 0 else s_pool1
                        mt = mpool.tile([128, NB, 128], BF, tag=f"msg{l}")
                        nc.gpsimd.dma_gather(
                            mt[:][:, :nb, :],
                            src_ap[l],
                            t_idx[l][:][:, b * NB * 8 : (b * NB + nb) * 8],
                            num_idxs=nb * 128,
                            num_idxs_reg=nb * 128,
                            elem_size=128,
                            elem_step=128,
                            single_packet=False,
                            queue_num=q,
                        )
                        st = spool.tile([128, NB * W], BF, tag=f"sg{l}")
                        nc.vector.tensor_tensor(
                            st[:].rearrange("p (c w) -> p c w", w=W)[:, :nb, :],
                            t_drel[l][:][:, b * NB : b * NB + nb][
                                :, :, None
                            ].broadcast_to([128, nb, W]),
                            t_iota[:][:, None, :].broadcast_to([128, nb, W]),
                            mybir.AluOpType.is_equal,
                        )
                        stiles[l].append(st)
                        htiles[l].append(mt)
                        loaded[l] += 1

                AGG = int(os.environ.get("K_AGG", "3"))
                if AGG == 1:
                    for l in (0, 1):
                        ensure(l, Cl[l] - 1)
                    return
                for t in range(NT):
                    chunks = []
                    if AGG >= 3:
                        for l in (0, 1):
                            for j in range(base[t][l], base[t][l] + chmax[t][l]):
                                chunks.append((l, int(j)))
                    ps = agg_psum.tile([128, CH], F32, tag="aggps")
                    nc.tensor.matmul(
                        ps[:],
                        t_id[:],
                        hp_tile[:][:, t * CH : (t + 1) * CH],
                        start=True,
                        stop=(not chunks),
                    )
                    nmm = len(chunks)
                    k = 0
                    for l, j in chunks:
                        ensure(l, j)
                        b, loc = divmod(j, NB)
                        s_ap = stiles[l][b][:][:, loc * W : (loc + 1) * W]
                        m_ap = htiles[l][b][:][
                            :, loc, l * CH : (l + 1) * CH
                        ]
                        k += 1
                        nc.tensor.matmul(
                            ps[:],
                            s_ap,
                            m_ap,
                            start=False,
                            stop=(k == nmm),
                        )
                    evict(t, ps)

            # ---- layer 1: transform x @ W1 -------------------------------
            hp1 = hp_pool.tile([128, NT * CH], F32, tag="hp")
            for t in range(NT):
                ps = tr_psum.tile([128, CH], F32, tag="trps")
                nc.tensor.matmul(
                    ps[:],
                    t_xT[:][:, t * 128 : (t + 1) * 128],
                    t_w1[:],
                    start=True,
                    stop=True,
                )
                nc.scalar.activation(
                    hp1[:][:, t * CH : (t + 1) * CH],
                    ps[:],
                    mybir.ActivationFunctionType.Copy,
                    bias=0.0,
                    scale=dinv_ap(t),
                )
            hpb1 = hpb_pool.tile([128, NT * CH], BF, tag="hpb")
            nc.scalar.copy(hpb1[:], hp1[:])
            store_hp(hpb1, 0)
            allgather(0)

            # ---- layer 1 aggregation + lrelu ------------------------------
            act1 = act_pool.tile([128, NT * CH], F32, tag="act")
            if STAGE < 3:
                nc.vector.memset(act1[:], 0.0)

            def evict_lrelu(act_tile, has_b, bkey):
                def _e(t, ps):
                    stg = stg_pool.tile([128, CH], F32, tag="stg")
                    if has_b:
                        nc.vector.scalar_tensor_tensor(
                            stg[:],
                            ps[:],
                            dinv_ap(t),
                            t_b[bkey][:],
                            mybir.AluOpType.mult,
                            mybir.AluOpType.add,
                        )
                    else:
                        nc.scalar.activation(
                            stg[:],
                            ps[:],
                            mybir.ActivationFunctionType.Copy,
                            bias=0.0,
                            scale=dinv_ap(t),
                        )
                    nc.vector.scalar_tensor_tensor(
                        act_tile[:][:, t * CH : (t + 1) * CH],
                        stg[:],
                        0.2,
                        stg[:],
                        mybir.AluOpType.mult,
                        mybir.AluOpType.max,
                    )
                return _e

            # per-window fused evict + next-layer transform: by the time the
            # last chunk matmul of a layer retires, 48/49 transforms are done
            # and the store+AllGather fires immediately.
            hp2 = hp_pool.tile([128, NT * CH], F32, tag="hp")
            hpb2 = hpb_pool.tile([128, NT * CH], BF, tag="hpb")
            ev1 = evict_lrelu(act1, has_b1, 1)

            def evict1_tr(t, ps):
                ev1(t, ps)
                psx = xt_psum.tile([CH, 128], F32, tag="trxt")
                nc.tensor.transpose(
                    psx[:], act1[:][:, t * CH : (t + 1) * CH], t_id[:]
                )
                xt = xt_pool.tile([CH, 128], F32, tag="xt")
                nc.scalar.copy(xt[:], psx[:])
                ps2 = tr_psum.tile([128, CH], F32, tag="trps")
                nc.tensor.matmul(ps2[:], xt[:], t_w2[:], start=True, stop=True)
                nc.scalar.activation(
                    hp2[:][:, t * CH : (t + 1) * CH],
                    ps2[:],
                    mybir.ActivationFunctionType.Copy,
                    bias=0.0,
                    scale=dinv_ap(t),
                )
                nc.scalar.copy(
                    hpb2[:][:, t * CH : (t + 1) * CH],
                    hp2[:][:, t * CH : (t + 1) * CH],
                )

            agg_layer(hp1, evict1_tr, 0)
            store_hp(hpb2, 1)
            allgather(1)

            # ---- layer 2 aggregation; layer-3 prescale fused --------------
            act2 = act_pool.tile([128, NT * CH], F32, tag="act")
            hp3 = hp_pool.tile([128, NT * CH], F32, tag="hp")
            hpb3 = hpb_pool.tile([128, NT * CH], BF, tag="hpb")
            ev2 = evict_lrelu(act2, has_b2, 2)

            def evict2_tr(t, ps):
                ev2(t, ps)
                nc.scalar.activation(
                    hp3[:][:, t * CH : (t + 1) * CH],
                    act2[:][:, t * CH : (t + 1) * CH],
                    mybir.ActivationFunctionType.Copy,
                    bias=0.0,
                    scale=dinv_ap(t),
                )
                nc.scalar.copy(
                    hpb3[:][:, t * CH : (t + 1) * CH],
                    hp3[:][:, t * CH : (t + 1) * CH],
                )

            agg_layer(hp2, evict2_tr, 1)
            store_hp(hpb3, 2)
            allgather(2)

            # ---- layer 3 aggregation; output transform + tanh fused -------
            agg3 = act_pool.tile([128, NT * CH], F32, tag="act")
            outsb = cpool.tile([128, NT * OUTC], F32, tag="outsb")

            def evict3_tr(t, ps):
                nc.scalar.activation(
                    agg3[:][:, t * CH : (t + 1) * CH],
                    ps[:],
                    mybir.ActivationFunctionType.Copy,
                    bias=0.0,
                    scale=dinv_ap(t),
                )
                psx = xt_psum.tile([CH, 128], F32, tag="trxt")
                nc.tensor.transpose(
                    psx[:], agg3[:][:, t * CH : (t + 1) * CH], t_id[:]
                )
                xt = xt_pool.tile([CH, 128], F32, tag="xt")
                nc.scalar.copy(xt[:], psx[:])
                ps3 = tr_psum.tile([128, OUTC], F32, tag="trps")
                nc.tensor.matmul(ps3[:], xt[:], t_w3[:], start=True, stop=True)
                o_ap = outsb[:][:, t * OUTC : (t + 1) * OUTC]
                if has_b3:
                    stg = stg_pool.tile([128, OUTC], F32, tag="stgo")
                    nc.vector.tensor_add(stg[:], ps3[:], t_b[3][:])
                    nc.scalar.activation(
                        o_ap, stg[:], mybir.ActivationFunctionType.Tanh
                    )
                else:
                    nc.scalar.activation(
                        o_ap, ps3[:], mybir.ActivationFunctionType.Tanh
                    )

            agg_layer(hp3, evict3_tr, 2)
            nc.sync.dma_start(out_d[:], outsb[:])

    nc.finalize()


def kernel(x, edge_index, W1, b1, W2, b2, W3, b3):
    global LAST_PERF
    x = np.asarray(x, np.float32)
    edge_index = np.asarray(edge_index)
    W1 = np.asarray(W1, np.float32)
    W2 = np.asarray(W2, np.float32)
    W3 = np.asarray(W3, np.float32)
    b1 = np.asarray(b1, np.float32)
    b2 = np.asarray(b2, np.float32)
    b3 = np.asarray(b3, np.float32)

    meta, per_core, newid = _prep(x, edge_index)
    has_b1 = bool(np.any(b1))
    has_b2 = bool(np.any(b2))
    has_b3 = bool(np.any(b3))

    if os.environ.get("BASS_TRACE"):
        _install_ntff_hook()

    nc = bacc.Bacc("TRN2", target_bir_lowering=False, debug=False,
                   num_devices=NCORES, num_swdge_queues=4)
    _build(nc, meta, has_b1, has_b2, has_b3)

    NT = meta["NT"]
    iota = np.broadcast_to(np.arange(W, dtype=np.int16), (128, W)).copy()
    ident = np.eye(128, dtype=np.float32)
    common = {
        "w1": W1, "w2": W2, "w3": W3, "ident": ident, "iota": iota,
    }
    if has_b1:
        common["b1b"] = np.broadcast_to(b1, (128, 64)).copy()
    if has_b2:
        common["b2b"] = np.broadcast_to(b2, (128, 64)).copy()
    if has_b3:
        common["b3b"] = np.broadcast_to(b3, (128, 16)).copy()

    in_maps = [{**per_core[c], **common} for c in range(NCORES)]
    res = run_bass_kernel_spmd(nc, in_maps, core_ids=list(range(NCORES)))
    LAST_PERF = res

    # reassemble: out[c] is [128, NT*16]; row t*128+p of core slice = [p, t*16:]
    N = meta["N"]
    NPCp = meta["NPCp"]
    full = np.empty((meta["Np"], 16), np.float32)
    for c in range(NCORES):
        o = res.results[c]["out"]  # [128, NT*16]
        full[c * NPCp : (c + 1) * NPCp] = (
            o.reshape(128, NT, 16).transpose(1, 0, 2).reshape(NPCp, 16)
        )
    out = np.empty((N, 16), np.float32)
    out[:] = full[newid]
    return out

